# revision 29
# baseline (speedup 1.0000x reference)
"""AppearanceDecoder Trainium2 kernel — 8-core data-parallel over batch.

Math (per batch b, per level l with feat F [Cin, D], conv weight w [256, Cin],
conv bias bias_l [256]):
    reference: fp = w @ F + bias; S = outq @ fp; A = softmax_d(S); q_l = A @ fp^T
    Since softmax is invariant to a per-row constant, the conv bias drops out of
    the scores (outq @ bias is constant over d), and sum_d A = 1 makes it a pure
    additive term in q_l:
        S  = (outq @ w) @ F                  (contract over Cin - feat natural layout)
        e  = exp(S - SHIFT); Z = sum_d e     (SHIFT const; softmax shift-invariant)
        q_l = (fpT^T @ e^T)^T / Z + bias_l   where fpT = F^T @ w^T  [D, 256]
    fpT is computed on-chip (contract over Cin - feat natural layout again), so
    feat is read from HBM exactly once and consumed by both matmuls.
Then aq = concat(q_0, q_1, q_2); 2-layer agg MLP; LayerNorm; 3-layer proj MLP.
The MLP chain runs in channel-on-partition (transposed) layout so biases fuse
into per-partition ACT copies; LayerNorm runs in query-on-partition layout.

All matmul-feeding tensors are float32r (full fp32 bits in DRAM; the PE rounds
internally - measured ~1.5e-4 matmul rel err, at bf16 throughput).

v4/v5: PE warm-up matmuls during the initial DMA fill (HAM un-throttle);
per-level aqT transposes folded into each level's tail; eT copies on ACT,
fpT copies on DVE.
v3: weights host-packed into [128, X] blocks so each loads as one DMA with
multi-KB contiguous lines (v2's 1KB/4B-line weight DMAs clogged the single DMA
queue for 37us before the first matmul). Weight packs are emitted mid-stream
(after a few feat tiles) so they never block the feat pipeline. Feat DMAs use
4KB lines (1024-wide double tiles). owT is computed lazily at each level start.
"""
import numpy as np
from contextlib import ExitStack

import concourse.bass as bass
import concourse.tile as tile
from concourse import bacc, mybir
from concourse.masks import make_identity

F32 = mybir.dt.float32
F32R = mybir.dt.float32r
AF = mybir.ActivationFunctionType

Q = 100
B = 8
C = 256
LEVELS = [(256, 16384), (512, 4096), (1024, 1024)]  # (Cin, D)
SHIFT = 88.0
N_CORES = 8

FEAT_BUFS = [6, 3, 1]  # [128, kc, 1024] tiles: 1MB / 2MB / 4MB each

# params0 layout (f32r, [128, 1224]): outqT[200] w0[512] w0T[512]
P0_OUTQT = (0, 200)
P0_W0 = (200, 712)
P0_W0T = (712, 1224)
# params1 (f32r, [128, 2048]): w1[1024] w1T[1024]
P1_W1 = (0, 1024)
P1_W1T = (1024, 2048)
# params2 (f32r, [128, 4096]): w2[2048] w2T[2048]
P2_W2 = (0, 2048)
P2_W2T = (2048, 4096)
# paramsE (f32r, [128, 3584]): aggw1T[1536] aggw2T[512] projw1T[512] projw2T[512] projw3T[512]
PE_OFF = [0, 1536, 2048, 2560, 3072, 3584]
# paramsf (f32, [128, 16]): bcat[6] aggb1[2] aggb2[2] projb1[2] projb2[2] projb3[2]
PF_OFF = [0, 6, 8, 10, 12, 14, 16]


def _mm(nc, out, lhsT, rhs, start, stop):
    nc.tensor.matmul(out, lhsT, rhs, start=start, stop=stop)


def build_graph():
    nc = bacc.Bacc("TRN2", target_bir_lowering=False, debug=False)

    feats = [
        nc.dram_tensor(f"feat{l}", [cin, d], F32R, kind="ExternalInput").ap()
        for l, (cin, d) in enumerate(LEVELS)
    ]
    params0 = nc.dram_tensor("params0", [128, 1224], F32R, kind="ExternalInput").ap()
    params1 = nc.dram_tensor("params1", [128, 2048], F32R, kind="ExternalInput").ap()
    params2 = nc.dram_tensor("params2", [128, 4096], F32R, kind="ExternalInput").ap()
    paramsE = nc.dram_tensor("paramsE", [128, 3584], F32R, kind="ExternalInput").ap()
    paramsf = nc.dram_tensor("paramsf", [128, 16], F32, kind="ExternalInput").ap()
    out_d = nc.dram_tensor("out", [C, Q], F32, kind="ExternalOutput").ap()

    with tile.TileContext(nc) as tc, ExitStack() as ctx:
        const = ctx.enter_context(tc.tile_pool(name="const", bufs=1))

        p0_sb = const.tile([128, 1224], F32R)
        nc.sync.dma_start(out=p0_sb, in_=params0)
        pf_sb = const.tile([128, 16], F32)
        nc.sync.dma_start(out=pf_sb, in_=paramsf)
        p1_sb = const.tile([128, 2048], F32R)
        p2_sb = const.tile([128, 4096], F32R)
        pE_sb = const.tile([128, 3584], F32R)

        identF = const.tile([128, 128], F32)
        make_identity(nc, identF)
        identR = const.tile([128, 128], F32R)
        nc.vector.tensor_copy(identR, identF)
        negc = const.tile([128, 1], F32)
        nc.vector.memset(negc, -SHIFT)
        aq = const.tile([Q, 3 * C], F32)

        outqT_sb = p0_sb[:, P0_OUTQT[0]:P0_OUTQT[1]].rearrange(
            "p (a q) -> p a q", a=2)
        w_sbs = [
            p0_sb[:, P0_W0[0]:P0_W0[1]].rearrange("p (a c) -> p a c", a=2),
            p1_sb[:, P1_W1[0]:P1_W1[1]].rearrange("p (a c) -> p a c", a=2),
            p2_sb[:, P2_W2[0]:P2_W2[1]].rearrange("p (a c) -> p a c", a=2),
        ]
        wT_sbs = [
            p0_sb[:, P0_W0T[0]:P0_W0T[1]].rearrange("p (j o) -> p j o", o=C),
            p1_sb[:, P1_W1T[0]:P1_W1T[1]].rearrange("p (j o) -> p j o", o=C),
            p2_sb[:, P2_W2T[0]:P2_W2T[1]].rearrange("p (j o) -> p j o", o=C),
        ]
        aggw1T_sb = pE_sb[:, PE_OFF[0]:PE_OFF[1]].rearrange("p (k o) -> p k o", o=C)
        aggw2T_sb = pE_sb[:, PE_OFF[1]:PE_OFF[2]].rearrange("p (k o) -> p k o", o=C)
        projwT_sb = [
            pE_sb[:, PE_OFF[2 + i]:PE_OFF[3 + i]].rearrange("p (k o) -> p k o", o=C)
            for i in range(3)
        ]
        bcat_sb = pf_sb[:, PF_OFF[0]:PF_OFF[1]]
        aggb1_sb = pf_sb[:, PF_OFF[1]:PF_OFF[2]]
        aggb2_sb = pf_sb[:, PF_OFF[2]:PF_OFF[3]]
        projb_sb = [pf_sb[:, PF_OFF[3 + i]:PF_OFF[4 + i]] for i in range(3)]

        owT_sbs = [
            const.tile([128, cin // 128, Q], F32R, name=f"owT{lvl}_sb")
            for lvl, (cin, _) in enumerate(LEVELS)
        ]

        with ExitStack() as mctx:
            pss = mctx.enter_context(tc.tile_pool(name="pss", bufs=3, space="PSUM"))
            pst = mctx.enter_context(tc.tile_pool(name="pst", bufs=1, space="PSUM"))
            psc = mctx.enter_context(tc.tile_pool(name="psc", bufs=3, space="PSUM"))
            psq = mctx.enter_context(tc.tile_pool(name="psq", bufs=1, space="PSUM"))

            # PE warm-up: ~24 back-to-back matmuls on the identity while the
            # first feat DMAs land; flips the HAM clock gate to 8/8 so real
            # matmuls start at 2.4 GHz.
            warm = pst.tile([128, 4, Q], F32R, name="warm", tag="tp")
            for i in range(30):
                nc.tensor.transpose(warm[:, i % 4, :], identR[:Q, :], identR[:Q, :Q])

            epool = mctx.enter_context(tc.tile_pool(name="e", bufs=4))
            etpool = mctx.enter_context(tc.tile_pool(name="et", bufs=4))
            fppool = mctx.enter_context(tc.tile_pool(name="fp", bufs=4))
            fpools = [
                mctx.enter_context(
                    tc.tile_pool(name=f"ft{lvl}", bufs=FEAT_BUFS[lvl])
                )
                for lvl in range(3)
            ]
            sums_t = [
                mctx.enter_context(tc.tile_pool(name=f"sums{lvl}", bufs=1)).tile(
                    [Q, LEVELS[lvl][1] // 512], F32, name=f"sums{lvl}"
                )
                for lvl in range(3)
            ]
            aqT = const.tile([128, 6, Q], F32R)
            ft2_pre = fpools[2].tile([128, 8, 1024], F32R, name="ft2_pre", tag="ft")

            for lvl, (cin, dd) in enumerate(LEVELS):
                kc = cin // 128
                nd2 = dd // 1024
                f_r = feats[lvl].rearrange("(j p) d -> p j d", p=128)
                w_sb, wT_sb, owT_sb = w_sbs[lvl], wT_sbs[lvl], owT_sbs[lvl]
                sums = sums_t[lvl]
                nc.vector.memset(sums, 0.0)

                # owT[c, q] = sum_o w[o, c] * outqT[o, q]
                for j in range(kc):
                    pw = pss.tile([128, Q], F32, name=f"pw{lvl}_{j}", tag="s")
                    for oc in range(2):
                        _mm(nc, pw, w_sb[:, oc, j * 128:(j + 1) * 128],
                            outqT_sb[:, oc, :], oc == 0, oc == 1)
                    nc.vector.tensor_copy(owT_sb[:, j, :], pw)

                qp = psq.tile([Q, C], F32, name=f"qp{lvl}", tag="qp")

                for n2 in range(nd2):
                    if lvl == 2:
                        ft = ft2_pre
                    else:
                        ft = fpools[lvl].tile(
                            [128, kc, 1024], F32R, name=f"ft{lvl}_{n2}", tag="ft"
                        )
                    if lvl == 2:
                        pass  # DMAs already issued during level 1
                    elif lvl == 0 and n2 == 0:
                        for j in range(kc):
                            for hh in range(2):
                                nc.sync.dma_start(
                                    out=ft[:, j, hh * 512:(hh + 1) * 512],
                                    in_=f_r[:, j, hh * 512:(hh + 1) * 512],
                                )
                    else:
                        for j in range(kc):
                            nc.sync.dma_start(
                                out=ft[:, j, :], in_=f_r[:, j, n2 * 1024:(n2 + 1) * 1024]
                            )
                    # stage the later weight packs behind the first feat tiles,
                    # in halves so feat DMAs interleave between them
                    if lvl == 0 and n2 == 2:
                        nc.sync.dma_start(out=p1_sb, in_=params1)
                    if lvl == 0 and 4 <= n2 <= 7:
                        qo = (n2 - 4) * 1024
                        nc.sync.dma_start(out=p2_sb[:, qo:qo + 1024],
                                          in_=params2[:, qo:qo + 1024])
                    if lvl == 0 and 8 <= n2 <= 11:
                        qo = (n2 - 8) * 896
                        nc.sync.dma_start(out=pE_sb[:, qo:qo + 896],
                                          in_=paramsE[:, qo:qo + 896])
                    # prefetch level2's single feat tile during level 1
                    if lvl == 1 and n2 in (1, 2):
                        f2r = feats[2].rearrange("(j p) d -> p j d", p=128)
                        for j in range(4 * (n2 - 1), 4 * n2):
                            nc.sync.dma_start(
                                out=ft2_pre[:, j, :], in_=f2r[:, j, :]
                            )

                    for h in range(2):
                        n = n2 * 2 + h
                        hof = h * 512
                        ps_s = pss.tile([Q, 512], F32, name=f"s{lvl}_{n}", tag="s")
                        for j in range(kc):
                            _mm(nc, ps_s, owT_sb[:, j, :],
                                ft[:, j, hof:hof + 512], j == 0, j == kc - 1)
                        e_sb = epool.tile([Q, 512], F32R, name=f"e{lvl}_{n}", tag="e")
                        nc.scalar.activation(
                            out=e_sb, in_=ps_s, func=AF.Exp,
                            bias=negc[:Q], scale=1.0, accum_out=sums[:, n:n + 1],
                        )
                        tp = pst.tile([128, 4, Q], F32R, name=f"tp{lvl}_{n}", tag="tp")
                        for m in range(4):
                            nc.tensor.transpose(
                                tp[:, m, :], e_sb[:, m * 128:(m + 1) * 128],
                                identR[:Q, :Q],
                            )
                        eT = etpool.tile(
                            [128, 4, Q], F32R, name=f"eT{lvl}_{n}", tag="eT"
                        )
                        nc.scalar.copy(out=eT, in_=tp)
                        for mh in range(2):
                            ps_c = psc.tile(
                                [128, 2, C], F32, name=f"c{lvl}_{n}_{mh}", tag="c"
                            )
                            for m2 in range(2):
                                m = mh * 2 + m2
                                for j in range(kc):
                                    _mm(nc, ps_c[:, m2, :],
                                        ft[:, j, hof + m * 128:hof + (m + 1) * 128],
                                        wT_sb[:, j, :], j == 0, j == kc - 1)
                            fpT = fppool.tile(
                                [128, 2, C], F32R, name=f"fpT{lvl}_{n}_{mh}",
                                tag="fpT"
                            )
                            nc.vector.tensor_copy(fpT, ps_c)
                            for m2 in range(2):
                                m = mh * 2 + m2
                                _mm(nc, qp, eT[:, m, :], fpT[:, m2, :],
                                    n == 0 and m == 0,
                                    n2 == nd2 - 1 and h == 1 and m == 3)

                zsum = const.tile([Q, 1], F32, name=f"zsum{lvl}")
                nc.vector.reduce_sum(out=zsum, in_=sums, axis=mybir.AxisListType.X)
                r_t = const.tile([Q, 1], F32, name=f"rt{lvl}")
                nc.vector.reciprocal(out=r_t, in_=zsum)
                nc.vector.tensor_scalar_mul(aq[:, lvl * C:(lvl + 1) * C], qp, r_t)
                # fold this level's aqT transposes + agg1 partial sums in now
                for kk in range(2):
                    k = 2 * lvl + kk
                    tpq = pst.tile([128, Q], F32, name=f"tpq{k}", tag="tp")
                    nc.tensor.transpose(
                        tpq, aq[:, k * 128:(k + 1) * 128], identF[:Q, :Q]
                    )
                    nc.scalar.activation(
                        out=aqT[:, k, :], in_=tpq, func=AF.Identity,
                        bias=bcat_sb[:, k:k + 1], scale=1.0,
                    )

        # ---- epilogue: agg MLP -> LN -> proj MLP, channel-on-partition ----
        with ExitStack() as ectx:
            ep = ectx.enter_context(tc.tile_pool(name="ep", bufs=1))
            psE = ectx.enter_context(tc.tile_pool(name="psE", bufs=3, space="PSUM"))

            def dense_T(src, w_sb, b_sb, func, out_dtype, nk, name):
                dst = ep.tile([128, 2, Q], out_dtype, name=name)
                for oc in range(2):
                    pz = psE.tile([128, Q], F32, name=f"{name}_p{oc}", tag="eps")
                    for k in range(nk):
                        _mm(nc, pz, w_sb[:, k, oc * 128:(oc + 1) * 128],
                            src[:, k, :], k == 0, k == nk - 1)
                    nc.scalar.activation(
                        out=dst[:, oc, :], in_=pz, func=func,
                        bias=b_sb[:, oc:oc + 1], scale=1.0,
                    )
                return dst

            z1T = dense_T(aqT, aggw1T_sb, aggb1_sb, AF.Relu, F32R, 6, "z1T")
            z2T = dense_T(z1T, aggw2T_sb, aggb2_sb, AF.Identity, F32R, 2, "z2T")

            z2 = ep.tile([Q, C], F32)
            for k in range(2):
                tpz = psE.tile([Q, 128], F32R, name=f"tpz{k}", tag="eps")
                nc.tensor.transpose(tpz, z2T[:, k, :], identR[:128, :128])
                nc.vector.tensor_copy(z2[:, k * 128:(k + 1) * 128], tpz)
            stats = ep.tile([Q, 6], F32)
            nc.vector.bn_stats(out=stats, in_=z2)
            mv = ep.tile([Q, 2], F32)
            nc.vector.bn_aggr(out=mv, in_=stats)
            eps_t = ep.tile([Q, 1], F32)
            nc.vector.memset(eps_t, 1e-5)
            sd = ep.tile([Q, 1], F32)
            nc.scalar.activation(out=sd, in_=mv[:, 1:2], func=AF.Sqrt,
                                 bias=eps_t, scale=1.0)
            rstd = ep.tile([Q, 1], F32)
            nc.vector.reciprocal(out=rstd, in_=sd)
            zn = ep.tile([Q, C], F32)
            nc.vector.tensor_scalar(
                out=zn, in0=z2, scalar1=mv[:, 0:1], scalar2=rstd,
                op0=mybir.AluOpType.subtract, op1=mybir.AluOpType.mult,
            )

            znT = ep.tile([128, 2, Q], F32R)
            for k in range(2):
                tpn = psE.tile([128, Q], F32, name=f"tpn{k}", tag="eps")
                nc.tensor.transpose(
                    tpn, zn[:, k * 128:(k + 1) * 128], identF[:Q, :Q]
                )
                nc.scalar.copy(out=znT[:, k, :], in_=tpn)

            zp1 = dense_T(znT, projwT_sb[0], projb_sb[0], AF.Relu, F32R, 2, "zp1")
            zp2 = dense_T(zp1, projwT_sb[1], projb_sb[1], AF.Relu, F32R, 2, "zp2")
            zp3 = dense_T(zp2, projwT_sb[2], projb_sb[2], AF.Identity, F32, 2, "zp3")
            nc.sync.dma_start(
                out=out_d.rearrange("(a p) q -> p a q", p=128), in_=zp3
            )

    nc.compile()
    return nc


_GRAPH = None


def _get_graph():
    global _GRAPH
    if _GRAPH is None:
        _GRAPH = build_graph()
    return _GRAPH


def _tile_p(a):
    """[r*128, K] -> [128, r*K] host pre-tiling (partition-major packing)."""
    r = a.shape[0] // 128
    return a.reshape(r, 128, -1).transpose(1, 0, 2).reshape(128, -1)


def _vec_p(v):
    """[r*128] -> [128, r]"""
    r = v.shape[0] // 128
    return v.reshape(r, 128).T


def make_in_maps(output, feat0, feat1, feat2,
                 w0, b0, w1, b1, w2, b2, ln_g, ln_b,
                 agg_w1, agg_b1, agg_w2, agg_b2,
                 proj_w1, proj_b1, proj_w2, proj_b2, proj_w3, proj_b3):
    f32 = np.float32
    c = lambda a: np.ascontiguousarray(a, dtype=f32)
    w0, w1, w2 = (np.asarray(x, f32) for x in (w0, w1, w2))
    p1 = c(np.concatenate([_tile_p(w1), _tile_p(np.ascontiguousarray(w1.T))], axis=1))
    p2 = c(np.concatenate([_tile_p(w2), _tile_p(np.ascontiguousarray(w2.T))], axis=1))
    lng_v = np.asarray(ln_g, f32)
    pw1g = np.asarray(proj_w1, f32) * lng_v[None, :]
    pE = c(np.concatenate(
        [_tile_p(np.ascontiguousarray(np.asarray(w, f32).T))
         for w in (agg_w1, agg_w2, pw1g, proj_w2, proj_w3)], axis=1))
    pf = c(np.concatenate(
        [_vec_p(np.asarray(v, f32)) for v in
         (np.concatenate([b0, b1, b2]), agg_b1, agg_b2,
          np.asarray(proj_w1, f32) @ np.asarray(ln_b, f32) + proj_b1,
          proj_b2, proj_b3)], axis=1))
    shared = {
        "params1": p1, "params2": p2, "paramsE": pE, "paramsf": pf,
    }
    feats = [feat0, feat1, feat2]
    in_maps = []
    for b in range(N_CORES):
        m = dict(shared)
        m["params0"] = c(np.concatenate(
            [_tile_p(np.ascontiguousarray(np.asarray(output, f32)[:, b, :].T)),
             _tile_p(w0), _tile_p(np.ascontiguousarray(w0.T))], axis=1))
        for l, (cin, d) in enumerate(LEVELS):
            m[f"feat{l}"] = c(feats[l][b].reshape(cin, d))
        in_maps.append(m)
    return in_maps


def kernel(output, feat0, feat1, feat2, output_mask,
           w0, b0, w1, b1, w2, b2, ln_g, ln_b,
           agg_w1, agg_b1, agg_w2, agg_b2,
           proj_w1, proj_b1, proj_w2, proj_b2, proj_w3, proj_b3,
           **_unused):
    from concourse.bass_utils import run_bass_kernel_spmd

    nc = _get_graph()
    in_maps = make_in_maps(
        output, feat0, feat1, feat2, w0, b0, w1, b1, w2, b2, ln_g, ln_b,
        agg_w1, agg_b1, agg_w2, agg_b2,
        proj_w1, proj_b1, proj_w2, proj_b2, proj_w3, proj_b3,
    )
    res = run_bass_kernel_spmd(nc, in_maps, core_ids=list(range(N_CORES)))
    return np.stack([res.results[b]["out"].T for b in range(N_CORES)], axis=1)


# revision 30
# speedup vs baseline: 1.0285x; 1.0285x over previous
"""AppearanceDecoder Trainium2 kernel — 8-core data-parallel over batch.

Math (per batch b, per level l with feat F [Cin, D], conv weight w [256, Cin],
conv bias bias_l [256]):
    reference: fp = w @ F + bias; S = outq @ fp; A = softmax_d(S); q_l = A @ fp^T
    Since softmax is invariant to a per-row constant, the conv bias drops out of
    the scores (outq @ bias is constant over d), and sum_d A = 1 makes it a pure
    additive term in q_l:
        S  = (outq @ w) @ F                  (contract over Cin - feat natural layout)
        e  = exp(S - SHIFT); Z = sum_d e     (SHIFT const; softmax shift-invariant)
        q_l = (fpT^T @ e^T)^T / Z + bias_l   where fpT = F^T @ w^T  [D, 256]
    fpT is computed on-chip (contract over Cin - feat natural layout again), so
    feat is read from HBM exactly once and consumed by both matmuls.
Then aq = concat(q_0, q_1, q_2); 2-layer agg MLP; LayerNorm; 3-layer proj MLP.
The MLP chain runs in channel-on-partition (transposed) layout so biases fuse
into per-partition ACT copies; LayerNorm runs in query-on-partition layout.

All matmul-feeding tensors are float32r (full fp32 bits in DRAM; the PE rounds
internally - measured ~1.5e-4 matmul rel err, at bf16 throughput).

v4/v5: PE warm-up matmuls during the initial DMA fill (HAM un-throttle);
per-level aqT transposes folded into each level's tail; eT copies on ACT,
fpT copies on DVE.
v3: weights host-packed into [128, X] blocks so each loads as one DMA with
multi-KB contiguous lines (v2's 1KB/4B-line weight DMAs clogged the single DMA
queue for 37us before the first matmul). Weight packs are emitted mid-stream
(after a few feat tiles) so they never block the feat pipeline. Feat DMAs use
4KB lines (1024-wide double tiles). owT is computed lazily at each level start.
"""
import numpy as np
from contextlib import ExitStack

import concourse.bass as bass
import concourse.tile as tile
from concourse import bacc, mybir
from concourse.masks import make_identity

F32 = mybir.dt.float32
F32R = mybir.dt.float32r
AF = mybir.ActivationFunctionType

Q = 100
B = 8
C = 256
LEVELS = [(256, 16384), (512, 4096), (1024, 1024)]  # (Cin, D)
SHIFT = 88.0
N_CORES = 8

FEAT_BUFS = [6, 3, 1]  # [128, kc, 1024] tiles: 1MB / 2MB / 4MB each

# params0 layout (f32r, [128, 1224]): outqT[200] w0[512] w0T[512]
P0_OUTQT = (0, 200)
P0_W0 = (200, 712)
P0_W0T = (712, 1224)
# params1 (f32r, [128, 2048]): w1[1024] w1T[1024]
P1_W1 = (0, 1024)
P1_W1T = (1024, 2048)
# params2 (f32r, [128, 4096]): w2[2048] w2T[2048]
P2_W2 = (0, 2048)
P2_W2T = (2048, 4096)
# paramsE (f32r, [128, 3584]): aggw1T[1536] aggw2T[512] projw1T[512] projw2T[512] projw3T[512]
PE_OFF = [0, 1536, 2048, 2560, 3072, 3584]
# paramsf (f32, [128, 16]): bcat[6] aggb1[2] aggb2[2] projb1[2] projb2[2] projb3[2]
PF_OFF = [0, 6, 8, 10, 12, 14, 16]


def _mm(nc, out, lhsT, rhs, start, stop):
    nc.tensor.matmul(out, lhsT, rhs, start=start, stop=stop)


def build_graph():
    nc = bacc.Bacc("TRN2", target_bir_lowering=False, debug=False)

    feats = [
        nc.dram_tensor(f"feat{l}", [cin, d], F32R, kind="ExternalInput").ap()
        for l, (cin, d) in enumerate(LEVELS)
    ]
    params0 = nc.dram_tensor("params0", [128, 1224], F32R, kind="ExternalInput").ap()
    params1 = nc.dram_tensor("params1", [128, 2048], F32R, kind="ExternalInput").ap()
    params2 = nc.dram_tensor("params2", [128, 4096], F32R, kind="ExternalInput").ap()
    paramsE = nc.dram_tensor("paramsE", [128, 3584], F32R, kind="ExternalInput").ap()
    paramsf = nc.dram_tensor("paramsf", [128, 16], F32, kind="ExternalInput").ap()
    out_d = nc.dram_tensor("out", [C, Q], F32, kind="ExternalOutput").ap()

    with tile.TileContext(nc) as tc, ExitStack() as ctx:
        const = ctx.enter_context(tc.tile_pool(name="const", bufs=1))

        p0_sb = const.tile([128, 1224], F32R)
        nc.sync.dma_start(out=p0_sb, in_=params0)
        pf_sb = const.tile([128, 16], F32)
        nc.sync.dma_start(out=pf_sb, in_=paramsf)
        p1_sb = const.tile([128, 2048], F32R)
        p2_sb = const.tile([128, 4096], F32R)
        pE_sb = const.tile([128, 3584], F32R)

        identF = const.tile([128, 128], F32)
        make_identity(nc, identF)
        identR = const.tile([128, 128], F32R)
        nc.vector.tensor_copy(identR, identF)
        negc = const.tile([128, 1], F32)
        nc.vector.memset(negc, -SHIFT)
        aq = const.tile([Q, 3 * C], F32)

        outqT_sb = p0_sb[:, P0_OUTQT[0]:P0_OUTQT[1]].rearrange(
            "p (a q) -> p a q", a=2)
        w_sbs = [
            p0_sb[:, P0_W0[0]:P0_W0[1]].rearrange("p (a c) -> p a c", a=2),
            p1_sb[:, P1_W1[0]:P1_W1[1]].rearrange("p (a c) -> p a c", a=2),
            p2_sb[:, P2_W2[0]:P2_W2[1]].rearrange("p (a c) -> p a c", a=2),
        ]
        wT_sbs = [
            p0_sb[:, P0_W0T[0]:P0_W0T[1]].rearrange("p (j o) -> p j o", o=C),
            p1_sb[:, P1_W1T[0]:P1_W1T[1]].rearrange("p (j o) -> p j o", o=C),
            p2_sb[:, P2_W2T[0]:P2_W2T[1]].rearrange("p (j o) -> p j o", o=C),
        ]
        aggw1T_sb = pE_sb[:, PE_OFF[0]:PE_OFF[1]].rearrange("p (k o) -> p k o", o=C)
        aggw2T_sb = pE_sb[:, PE_OFF[1]:PE_OFF[2]].rearrange("p (k o) -> p k o", o=C)
        projwT_sb = [
            pE_sb[:, PE_OFF[2 + i]:PE_OFF[3 + i]].rearrange("p (k o) -> p k o", o=C)
            for i in range(3)
        ]
        bcat_sb = pf_sb[:, PF_OFF[0]:PF_OFF[1]]
        aggb1_sb = pf_sb[:, PF_OFF[1]:PF_OFF[2]]
        aggb2_sb = pf_sb[:, PF_OFF[2]:PF_OFF[3]]
        projb_sb = [pf_sb[:, PF_OFF[3 + i]:PF_OFF[4 + i]] for i in range(3)]

        owT_sbs = [
            const.tile([128, cin // 128, Q], F32R, name=f"owT{lvl}_sb")
            for lvl, (cin, _) in enumerate(LEVELS)
        ]

        with ExitStack() as mctx:
            pss = mctx.enter_context(tc.tile_pool(name="pss", bufs=3, space="PSUM"))
            pst = mctx.enter_context(tc.tile_pool(name="pst", bufs=1, space="PSUM"))
            psc = mctx.enter_context(tc.tile_pool(name="psc", bufs=3, space="PSUM"))
            psq = mctx.enter_context(tc.tile_pool(name="psq", bufs=1, space="PSUM"))

            # PE warm-up: ~24 back-to-back matmuls on the identity while the
            # first feat DMAs land; flips the HAM clock gate to 8/8 so real
            # matmuls start at 2.4 GHz.
            warm = pst.tile([128, 4, Q], F32R, name="warm", tag="tp")
            for i in range(30):
                nc.tensor.transpose(warm[:, i % 4, :], identR[:Q, :], identR[:Q, :Q])

            epool = mctx.enter_context(tc.tile_pool(name="e", bufs=4))
            etpool = mctx.enter_context(tc.tile_pool(name="et", bufs=4))
            fppool = mctx.enter_context(tc.tile_pool(name="fp", bufs=4))
            fpools = [
                mctx.enter_context(
                    tc.tile_pool(name=f"ft{lvl}", bufs=FEAT_BUFS[lvl])
                )
                for lvl in range(3)
            ]
            sums_t = [
                mctx.enter_context(tc.tile_pool(name=f"sums{lvl}", bufs=1)).tile(
                    [Q, LEVELS[lvl][1] // 512], F32, name=f"sums{lvl}"
                )
                for lvl in range(3)
            ]
            aqT = const.tile([128, 6, Q], F32R)
            ft2_pre = fpools[2].tile([128, 8, 1024], F32R, name="ft2_pre", tag="ft")

            for lvl, (cin, dd) in enumerate(LEVELS):
                kc = cin // 128
                nd2 = dd // 1024
                f_r = feats[lvl].rearrange("(j p) d -> p j d", p=128)
                w_sb, wT_sb, owT_sb = w_sbs[lvl], wT_sbs[lvl], owT_sbs[lvl]
                sums = sums_t[lvl]
                nc.vector.memset(sums, 0.0)

                # owT[c, q] = sum_o w[o, c] * outqT[o, q]
                for j in range(kc):
                    pw = pss.tile([128, Q], F32, name=f"pw{lvl}_{j}", tag="s")
                    for oc in range(2):
                        _mm(nc, pw, w_sb[:, oc, j * 128:(j + 1) * 128],
                            outqT_sb[:, oc, :], oc == 0, oc == 1)
                    nc.vector.tensor_copy(owT_sb[:, j, :], pw)

                qp = psq.tile([Q, C], F32, name=f"qp{lvl}", tag="qp")

                for n2 in range(nd2):
                    if lvl == 2:
                        ft = ft2_pre
                    else:
                        ft = fpools[lvl].tile(
                            [128, kc, 1024], F32R, name=f"ft{lvl}_{n2}", tag="ft"
                        )
                    if lvl == 2:
                        pass  # DMAs already issued during level 1
                    elif lvl == 0 and n2 == 0:
                        for j in range(kc):
                            for hh in range(2):
                                nc.sync.dma_start(
                                    out=ft[:, j, hh * 512:(hh + 1) * 512],
                                    in_=f_r[:, j, hh * 512:(hh + 1) * 512],
                                )
                    else:
                        for j in range(kc):
                            nc.sync.dma_start(
                                out=ft[:, j, :], in_=f_r[:, j, n2 * 1024:(n2 + 1) * 1024]
                            )
                    # stage the later weight packs behind the first feat tiles,
                    # in halves so feat DMAs interleave between them
                    if lvl == 0 and n2 == 2:
                        nc.sync.dma_start(out=p1_sb, in_=params1)
                    if lvl == 0 and 4 <= n2 <= 7:
                        qo = (n2 - 4) * 1024
                        nc.sync.dma_start(out=p2_sb[:, qo:qo + 1024],
                                          in_=params2[:, qo:qo + 1024])
                    if lvl == 0 and 8 <= n2 <= 11:
                        qo = (n2 - 8) * 896
                        nc.sync.dma_start(out=pE_sb[:, qo:qo + 896],
                                          in_=paramsE[:, qo:qo + 896])
                    # prefetch level2's single feat tile during level 1
                    if lvl == 1 and n2 in (2, 3):
                        f2r = feats[2].rearrange("(j p) d -> p j d", p=128)
                        for j in range(4 * (n2 - 2), 4 * (n2 - 1)):
                            nc.sync.dma_start(
                                out=ft2_pre[:, j, :], in_=f2r[:, j, :]
                            )

                    for h in range(2):
                        n = n2 * 2 + h
                        hof = h * 512
                        ps_s = pss.tile([Q, 512], F32, name=f"s{lvl}_{n}", tag="s")
                        for j in range(kc):
                            _mm(nc, ps_s, owT_sb[:, j, :],
                                ft[:, j, hof:hof + 512], j == 0, j == kc - 1)
                        e_sb = epool.tile([Q, 512], F32R, name=f"e{lvl}_{n}", tag="e")
                        nc.scalar.activation(
                            out=e_sb, in_=ps_s, func=AF.Exp,
                            bias=negc[:Q], scale=1.0, accum_out=sums[:, n:n + 1],
                        )
                        tp = pst.tile([128, 4, Q], F32R, name=f"tp{lvl}_{n}", tag="tp")
                        for m in range(4):
                            nc.tensor.transpose(
                                tp[:, m, :], e_sb[:, m * 128:(m + 1) * 128],
                                identR[:Q, :Q],
                            )
                        eT = etpool.tile(
                            [128, 4, Q], F32R, name=f"eT{lvl}_{n}", tag="eT"
                        )
                        nc.scalar.copy(out=eT, in_=tp)
                        for mh in range(2):
                            ps_c = psc.tile(
                                [128, 2, C], F32, name=f"c{lvl}_{n}_{mh}", tag="c"
                            )
                            for m2 in range(2):
                                m = mh * 2 + m2
                                for j in range(kc):
                                    _mm(nc, ps_c[:, m2, :],
                                        ft[:, j, hof + m * 128:hof + (m + 1) * 128],
                                        wT_sb[:, j, :], j == 0, j == kc - 1)
                            fpT = fppool.tile(
                                [128, 2, C], F32R, name=f"fpT{lvl}_{n}_{mh}",
                                tag="fpT"
                            )
                            nc.vector.tensor_copy(fpT, ps_c)
                            for m2 in range(2):
                                m = mh * 2 + m2
                                _mm(nc, qp, eT[:, m, :], fpT[:, m2, :],
                                    n == 0 and m == 0,
                                    n2 == nd2 - 1 and h == 1 and m == 3)

                zsum = const.tile([Q, 1], F32, name=f"zsum{lvl}")
                nc.vector.reduce_sum(out=zsum, in_=sums, axis=mybir.AxisListType.X)
                r_t = const.tile([Q, 1], F32, name=f"rt{lvl}")
                nc.vector.reciprocal(out=r_t, in_=zsum)
                nc.vector.tensor_scalar_mul(aq[:, lvl * C:(lvl + 1) * C], qp, r_t)
                # fold this level's aqT transposes + agg1 partial sums in now
                for kk in range(2):
                    k = 2 * lvl + kk
                    tpq = pst.tile([128, Q], F32, name=f"tpq{k}", tag="tp")
                    nc.tensor.transpose(
                        tpq, aq[:, k * 128:(k + 1) * 128], identF[:Q, :Q]
                    )
                    nc.scalar.activation(
                        out=aqT[:, k, :], in_=tpq, func=AF.Identity,
                        bias=bcat_sb[:, k:k + 1], scale=1.0,
                    )

        # ---- epilogue: agg MLP -> LN -> proj MLP, channel-on-partition ----
        with ExitStack() as ectx:
            ep = ectx.enter_context(tc.tile_pool(name="ep", bufs=1))
            psE = ectx.enter_context(tc.tile_pool(name="psE", bufs=3, space="PSUM"))

            def dense_T(src, w_sb, b_sb, func, out_dtype, nk, name):
                dst = ep.tile([128, 2, Q], out_dtype, name=name)
                for oc in range(2):
                    pz = psE.tile([128, Q], F32, name=f"{name}_p{oc}", tag="eps")
                    for k in range(nk):
                        _mm(nc, pz, w_sb[:, k, oc * 128:(oc + 1) * 128],
                            src[:, k, :], k == 0, k == nk - 1)
                    nc.scalar.activation(
                        out=dst[:, oc, :], in_=pz, func=func,
                        bias=b_sb[:, oc:oc + 1], scale=1.0,
                    )
                return dst

            z1T = dense_T(aqT, aggw1T_sb, aggb1_sb, AF.Relu, F32R, 6, "z1T")
            z2T = dense_T(z1T, aggw2T_sb, aggb2_sb, AF.Identity, F32R, 2, "z2T")

            z2 = ep.tile([Q, C], F32)
            for k in range(2):
                tpz = psE.tile([Q, 128], F32R, name=f"tpz{k}", tag="eps")
                nc.tensor.transpose(tpz, z2T[:, k, :], identR[:128, :128])
                nc.vector.tensor_copy(z2[:, k * 128:(k + 1) * 128], tpz)
            stats = ep.tile([Q, 6], F32)
            nc.vector.bn_stats(out=stats, in_=z2)
            mv = ep.tile([Q, 2], F32)
            nc.vector.bn_aggr(out=mv, in_=stats)
            eps_t = ep.tile([Q, 1], F32)
            nc.vector.memset(eps_t, 1e-5)
            sd = ep.tile([Q, 1], F32)
            nc.scalar.activation(out=sd, in_=mv[:, 1:2], func=AF.Sqrt,
                                 bias=eps_t, scale=1.0)
            rstd = ep.tile([Q, 1], F32)
            nc.vector.reciprocal(out=rstd, in_=sd)
            zn = ep.tile([Q, C], F32)
            nc.vector.tensor_scalar(
                out=zn, in0=z2, scalar1=mv[:, 0:1], scalar2=rstd,
                op0=mybir.AluOpType.subtract, op1=mybir.AluOpType.mult,
            )

            znT = ep.tile([128, 2, Q], F32R)
            for k in range(2):
                tpn = psE.tile([128, Q], F32, name=f"tpn{k}", tag="eps")
                nc.tensor.transpose(
                    tpn, zn[:, k * 128:(k + 1) * 128], identF[:Q, :Q]
                )
                nc.scalar.copy(out=znT[:, k, :], in_=tpn)

            zp1 = dense_T(znT, projwT_sb[0], projb_sb[0], AF.Relu, F32R, 2, "zp1")
            zp2 = dense_T(zp1, projwT_sb[1], projb_sb[1], AF.Relu, F32R, 2, "zp2")
            zp3 = dense_T(zp2, projwT_sb[2], projb_sb[2], AF.Identity, F32, 2, "zp3")
            nc.sync.dma_start(
                out=out_d.rearrange("(a p) q -> p a q", p=128), in_=zp3
            )

    nc.compile()
    return nc


_GRAPH = None


def _get_graph():
    global _GRAPH
    if _GRAPH is None:
        _GRAPH = build_graph()
    return _GRAPH


def _tile_p(a):
    """[r*128, K] -> [128, r*K] host pre-tiling (partition-major packing)."""
    r = a.shape[0] // 128
    return a.reshape(r, 128, -1).transpose(1, 0, 2).reshape(128, -1)


def _vec_p(v):
    """[r*128] -> [128, r]"""
    r = v.shape[0] // 128
    return v.reshape(r, 128).T


def make_in_maps(output, feat0, feat1, feat2,
                 w0, b0, w1, b1, w2, b2, ln_g, ln_b,
                 agg_w1, agg_b1, agg_w2, agg_b2,
                 proj_w1, proj_b1, proj_w2, proj_b2, proj_w3, proj_b3):
    f32 = np.float32
    c = lambda a: np.ascontiguousarray(a, dtype=f32)
    w0, w1, w2 = (np.asarray(x, f32) for x in (w0, w1, w2))
    p1 = c(np.concatenate([_tile_p(w1), _tile_p(np.ascontiguousarray(w1.T))], axis=1))
    p2 = c(np.concatenate([_tile_p(w2), _tile_p(np.ascontiguousarray(w2.T))], axis=1))
    lng_v = np.asarray(ln_g, f32)
    pw1g = np.asarray(proj_w1, f32) * lng_v[None, :]
    pE = c(np.concatenate(
        [_tile_p(np.ascontiguousarray(np.asarray(w, f32).T))
         for w in (agg_w1, agg_w2, pw1g, proj_w2, proj_w3)], axis=1))
    pf = c(np.concatenate(
        [_vec_p(np.asarray(v, f32)) for v in
         (np.concatenate([b0, b1, b2]), agg_b1, agg_b2,
          np.asarray(proj_w1, f32) @ np.asarray(ln_b, f32) + proj_b1,
          proj_b2, proj_b3)], axis=1))
    shared = {
        "params1": p1, "params2": p2, "paramsE": pE, "paramsf": pf,
    }
    feats = [feat0, feat1, feat2]
    in_maps = []
    for b in range(N_CORES):
        m = dict(shared)
        m["params0"] = c(np.concatenate(
            [_tile_p(np.ascontiguousarray(np.asarray(output, f32)[:, b, :].T)),
             _tile_p(w0), _tile_p(np.ascontiguousarray(w0.T))], axis=1))
        for l, (cin, d) in enumerate(LEVELS):
            m[f"feat{l}"] = c(feats[l][b].reshape(cin, d))
        in_maps.append(m)
    return in_maps


def kernel(output, feat0, feat1, feat2, output_mask,
           w0, b0, w1, b1, w2, b2, ln_g, ln_b,
           agg_w1, agg_b1, agg_w2, agg_b2,
           proj_w1, proj_b1, proj_w2, proj_b2, proj_w3, proj_b3,
           **_unused):
    from concourse.bass_utils import run_bass_kernel_spmd

    nc = _get_graph()
    in_maps = make_in_maps(
        output, feat0, feat1, feat2, w0, b0, w1, b1, w2, b2, ln_g, ln_b,
        agg_w1, agg_b1, agg_w2, agg_b2,
        proj_w1, proj_b1, proj_w2, proj_b2, proj_w3, proj_b3,
    )
    res = run_bass_kernel_spmd(nc, in_maps, core_ids=list(range(N_CORES)))
    return np.stack([res.results[b]["out"].T for b in range(N_CORES)], axis=1)
